# revision 1
# baseline (speedup 1.0000x reference)
"""HMM window log-likelihood on 8 NeuronCores (data-parallel over batch).

Math: reference computes, per batch column b,
    y[b] = exp(logsumexp_i x_T[b,i]),  x via log-space forward recursion.
Equivalently in linear space with row-normalized transition matrices
W_t = exp(w[t-1]) / rowsum, emission table L = softmax(distros, axis=1):
    y[b] = 1^T diag(em_T) W_T ... diag(em_1) W_1 em_0
We evaluate it as a BACKWARD recursion (avoids transposing W on device):
    beta_L = 1;  beta_{t-1} = W_t^T (em_t . beta_t)
    y[b] = sum_i em_0[i,b] beta_0[i,b]
with per-step rescale factors g_t (host-computed from column 0, f64) folded
into recipSg[:,t] = g_t / rowsum_t to keep everything in bf16/f32 range.
em_t[i,b] = L[i, bin(b,t)] is computed on the PE as dLT^T @ G_t where
dL[i,k] = L[i,k]-L[i,k-1] and G_t[k,b] = [bin(b,t) >= k] (0/1 indicators).
Device returns colsum[b] = y[b] * prod(g); host: lnY = log(colsum)+C, y=exp.
The true lnY is ~ -584.6 for these inputs, so y underflows f32 to 0.0 —
exactly matching the reference (which also underflows in f32).
"""
import sys, os
for p in ("/opt/trn_rl_repo",):
    if p not in sys.path:
        sys.path.insert(0, p)
import numpy as np
import ml_dtypes

from concourse import bass, bacc, mybir
from concourse.tile import TileContext
from concourse.bass_utils import run_bass_kernel_spmd

W, L, B, NB = 128, 256, 4096, 10
NCORES = 8
BC = B // NCORES          # 512 batch cols per core
BH = BC // 2              # two half-chains of 256
TBLK = 16                 # G streaming block (t's per DMA)

LAST_LNY = None           # debug: device-derived lnY per batch col
LAST_RESULTS = None       # debug: raw BassKernelResults

_CACHED = None            # (nc,) build cache


def _build_nc():
    nc = bacc.Bacc("TRN2", target_bir_lowering=False, debug=False,
                   num_devices=NCORES)
    bf16, f32 = mybir.dt.bfloat16, mybir.dt.float32

    wt = nc.dram_tensor("wt", [W, L - 1, W], bf16, kind="ExternalInput")
    dlt = nc.dram_tensor("dlt", [NB, W], bf16, kind="ExternalInput")
    rsg = nc.dram_tensor("rsg", [W, L], f32, kind="ExternalInput")
    g10 = nc.dram_tensor("g10", [NB, L, BC], bf16, kind="ExternalInput")
    ones = nc.dram_tensor("ones", [W, 1], bf16, kind="ExternalInput")
    colsum = nc.dram_tensor("colsum", [1, BC], f32, kind="ExternalOutput")

    Copy = mybir.ActivationFunctionType.Copy

    with TileContext(nc) as tc:
        with tc.sbuf_pool(name="sb", bufs=2) as sb, \
                tc.psum_pool(name="ps", bufs=2) as ps:
            dlt_sb = sb.tile([NB, W], bf16, bufs=1)
            nc.sync.dma_start(dlt_sb, dlt.ap())
            rsg_sb = sb.tile([W, L], f32, bufs=1)
            nc.sync.dma_start(rsg_sb, rsg.ap())
            ones_sb = sb.tile([W, 1], bf16, bufs=1)
            nc.sync.dma_start(ones_sb, ones.ap())

            # all 255 transition matrices resident; chunked DMAs in backward
            # order so the scan can start as soon as the tail chunk lands
            wt_sb = sb.tile([W, L - 1, W], bf16, bufs=1)
            for cc in range((L - 1 + 7) // 8 - 1, -1, -1):
                t0 = cc * 8
                cnt = min(8, L - 1 - t0)
                nc.sync.dma_start(wt_sb[:, t0:t0 + cnt, :],
                                  wt.ap()[:, t0:t0 + cnt, :])

            cs_ps = None
            beta_ps = [None, None]
            for blk in range(L // TBLK - 1, -1, -1):
                g_sb = sb.tile([NB, TBLK, BC], bf16, tag="g", bufs=3)
                nc.sync.dma_start(
                    g_sb, g10.ap()[:, blk * TBLK:(blk + 1) * TBLK, :])
                for ti in range(TBLK - 1, -1, -1):
                    t = blk * TBLK + ti
                    for h in (0, 1):
                        em_ps = ps.tile([W, BH], f32, tag=f"em{h}", bufs=2)
                        nc.tensor.matmul(
                            em_ps, dlt_sb,
                            g_sb[:, ti, h * BH:(h + 1) * BH],
                            start=True, stop=True)
                        em_sb = sb.tile([W, BH], bf16, tag=f"emsb{h}", bufs=3)
                        nc.scalar.activation(em_sb, em_ps, Copy,
                                             scale=rsg_sb[:, t:t + 1])
                        if t == L - 1:
                            c_sb = em_sb
                        else:
                            c_sb = sb.tile([W, BH], bf16, tag=f"c{h}", bufs=3)
                            nc.vector.tensor_mul(c_sb, beta_ps[h], em_sb)
                        if t > 0:
                            b_ps = ps.tile([W, BH], f32, tag=f"b{h}", bufs=2)
                            nc.tensor.matmul(b_ps, wt_sb[:, t - 1, :], c_sb,
                                             start=True, stop=True)
                            beta_ps[h] = b_ps
                        else:
                            if cs_ps is None:
                                cs_ps = ps.tile([1, BC], f32, tag="em0",
                                                bufs=2)
                            nc.tensor.matmul(cs_ps[:, h * BH:(h + 1) * BH],
                                             ones_sb, c_sb,
                                             start=True, stop=True)

            cs_sb = sb.tile([1, BC], f32, bufs=1)
            nc.vector.tensor_copy(cs_sb, cs_ps)
            nc.sync.dma_start(colsum.ap(), cs_sb)
    nc.compile()
    return nc


def _host_prep(data, input_distros, dense_layer_weights):
    f64 = np.float64
    we = np.exp(dense_layer_weights.astype(f64))           # (255,W,W)
    rowsum = we.sum(axis=2)                                # (255,W)
    recip = 1.0 / rowsum
    d = input_distros.astype(f64)
    d = d - d.max(axis=1, keepdims=True)
    e = np.exp(d)
    Ll = e / e.sum(axis=1, keepdims=True)                  # (W,NB) softmax rows
    # bins exactly as reference: floor(v / 0.1) in f32
    bins = np.minimum(NB - 1, np.floor(
        data / np.float32(0.1)).astype(np.int32))          # (B,L)

    # column-0 f64 backward pass -> per-step rescale g_t, offset C
    beta = np.ones(W, dtype=f64)
    Cacc = 0.0
    g = np.ones(L, dtype=f64)
    for t in range(L - 1, 0, -1):
        c = Ll[np.arange(W), bins[0, t]] * beta * recip[t - 1]
        tmp = we[t - 1].T @ c
        f = tmp.max()
        g[t] = 1.0 / f
        Cacc += np.log(f)
        beta = tmp * g[t]

    rsg = np.ones((W, L), dtype=np.float32)
    rsg[:, 1:] = (recip.T * g[None, 1:]).astype(np.float32)

    dL = Ll.copy()
    dL[:, 1:] -= Ll[:, :-1]
    dlt = np.ascontiguousarray(dL.T).astype(ml_dtypes.bfloat16)  # (NB,W)

    wt = np.ascontiguousarray(
        we.transpose(1, 0, 2)).astype(ml_dtypes.bfloat16)  # (W,255,W)

    # G[k,t,b] = [bins[b,t] >= k]   (G[0] == 1)
    g10 = (bins.T[None, :, :] >= np.arange(NB)[:, None, None]
           ).astype(ml_dtypes.bfloat16)                    # (NB,L,B)
    ones_v = np.ones((W, 1), dtype=ml_dtypes.bfloat16)
    return wt, dlt, rsg, g10, ones_v, Cacc


def kernel(data, input_distros, dense_layer_weights):
    global LAST_LNY, LAST_RESULTS, _CACHED
    wt, dlt, rsg, g10, ones_v, Cacc = _host_prep(
        np.asarray(data), np.asarray(input_distros),
        np.asarray(dense_layer_weights))

    if _CACHED is None:
        _CACHED = _build_nc()
    nc = _CACHED

    in_maps = []
    for c in range(NCORES):
        in_maps.append({
            "wt": wt, "dlt": dlt, "rsg": rsg, "ones": ones_v,
            "g10": np.ascontiguousarray(g10[:, :, c * BC:(c + 1) * BC]),
        })
    res = run_bass_kernel_spmd(
        nc, in_maps, core_ids=list(range(NCORES)),
        trace=bool(int(os.environ.get("KERNEL_TRACE", "0"))))
    LAST_RESULTS = res
    cs = np.concatenate([res.results[c]["colsum"].reshape(-1)
                         for c in range(NCORES)])           # (B,)
    lnY = np.log(cs.astype(np.float64)) + Cacc
    LAST_LNY = lnY
    y = np.exp(lnY).astype(np.float32).reshape(B, 1)
    return y



# revision 6
# speedup vs baseline: 1.8066x; 1.8066x over previous
"""HMM window log-likelihood on 8 NeuronCores (data-parallel over batch).

Math: reference computes, per batch column b,
    y[b] = exp(logsumexp_i x_T[b,i]),  x via log-space forward recursion.
Equivalently in linear space with row-normalized transition matrices
W_t = exp(w[t-1]) / rowsum, emission table L = softmax(distros, axis=1):
    y[b] = 1^T diag(em_T) W_T ... diag(em_1) W_1 em_0
Evaluated as a BACKWARD recursion:
    c_t = em_t . beta_t;  beta_{t-1} = W_t^T c_t;  colsum = 1^T c_0
with per-step rescale factors g_t (host f64, from column 0) folded into the
host-precomputed scaled emissions emsc[i,t,b] = L[i,bin(b,t)] * rsg[i,t].

Device structure (per core, BC=512 batch cols):
 - The 255-step serial chain is split into 6 SEGMENTS run as independent
   chains; each lower segment starts K steps early from beta=1 ("burn-in").
   The HMM forgets initial direction geometrically, so after K steps the
   burn-in beta is proportional to the true beta; the unknown per-column
   factor cancels exactly via junction sums J = 1^T c computed by BOTH
   chains at the junction step:  lnY += log(J_upper) - log(J_burnin).
 - Chains run in 3 lockstep PAIRS so each per-step op is one wide
   [128, 1024] instruction (2 chains x 512 cols).
 - Emissions arrive pre-scaled from host via DMA (bf16, SBUF) - no scalar
   activation and no K=10 emission matmuls (which bottlenecked v1).
 - Per-step c = em . beta with beta f32 in PSUM; lanes:
     pair 0 (direct): DVE tensor_mul(c, beta_psum_f32, em_bf16)  (1x mode)
     pairs 1,2 (cast): ScalarE casts beta f32->bf16, then DVE runs the
     multiply all-bf16/SBUF which triggers the DVE 2x_1P perf mode.
Device returns colsum + junction sums; host combines in f64:
lnY ~ -584.6 so y underflows f32 to 0.0, matching the reference exactly.
"""
import sys, os
for p in ("/opt/trn_rl_repo",):
    if p not in sys.path:
        sys.path.insert(0, p)
import numpy as np
import ml_dtypes

from concourse import bass, bacc, mybir
from concourse.tile import TileContext
from concourse.bass_utils import run_bass_kernel_spmd

W, L, B, NB = 128, 256, 4096, 10
NCORES = 8
BC = B // NCORES          # 512 batch cols per core
K_BURN = 10               # burn-in steps per junction
TBLK = 4                  # em streaming block (steps per DMA)

# Segment boundaries: 6 chains in 3 pairs, lockstep within a pair.
# chain s covers real t in [MLO[s], MHI[s]); burn-in K above MHI (top chain
# none).  Pair lanes: pair 2 = "direct" (DVE from PSUM), pairs 0,1 = "cast"
# (ACT f32->bf16 + DVE 2x).  Direct pair gets the top segments.
M_BOUND = [0, 43, 86, 129, 172, 209, 256]   # m0..m6
PAIRS = [(1, 2), (3, 4), (5, 6)]            # chain ids (1-based)
PAIR_LANE = ["cast", "cast", "direct"]
CHAIN_THI = {s: (M_BOUND[s] + K_BURN - 1 if s < 6 else 255) for s in range(1, 7)}
CHAIN_TLO = {s: M_BOUND[s - 1] for s in range(1, 7)}
PAIR_STEPS = [CHAIN_THI[a] - CHAIN_TLO[a] + 1 for a, _ in PAIRS]
for pi, (a, b) in enumerate(PAIRS):
    assert CHAIN_THI[a] - CHAIN_TLO[a] == CHAIN_THI[b] - CHAIN_TLO[b], (pi, a, b)

LAST_LNY = None           # debug: device-derived lnY per batch col
LAST_RESULTS = None       # debug: raw BassKernelResults

_CACHED = None


def _build_nc():
    nc = bacc.Bacc("TRN2", target_bir_lowering=False, debug=False,
                   num_devices=NCORES)
    bf16, f32 = mybir.dt.bfloat16, mybir.dt.float32
    Copy = mybir.ActivationFunctionType.Copy

    wt = nc.dram_tensor("wt", [W, L - 1, W], bf16, kind="ExternalInput")
    ems = [nc.dram_tensor(f"em{pi}", [W, PAIR_STEPS[pi], 2 * BC], bf16,
                          kind="ExternalInput") for pi in range(3)]
    ones = nc.dram_tensor("ones", [W, 1], bf16, kind="ExternalInput")
    juncs = nc.dram_tensor("juncs", [12, BC], f32, kind="ExternalOutput")

    with TileContext(nc) as tc:
        with tc.sbuf_pool(name="sb", bufs=2) as sb, \
                tc.psum_pool(name="ps", bufs=2) as ps:
            ones_sb = sb.tile([W, 1], bf16, bufs=1)
            nc.sync.dma_start(ones_sb, ones.ap())

            # transition matrices resident; per-chain backward chunks so
            # every chain's first weights land early
            wt_sb = sb.tile([W, L - 1, W], bf16, bufs=1)
            chunks = []   # (order_key, aligned_block_index)
            for s in range(1, 7):
                lo = max(CHAIN_TLO[s] - 1, 0)
                hi = CHAIN_THI[s] - 1          # MM at step t uses wt[t-1]
                blks = list(range(hi // 8, lo // 8 - 1, -1))
                for oi, bi in enumerate(blks):
                    chunks.append((oi, bi))
            chunks.sort()
            seen = set()
            for _, bi in chunks:
                if bi in seen:
                    continue
                seen.add(bi)
                t0 = bi * 8
                cnt = min(8, L - 1 - t0)
                nc.sync.dma_start(wt_sb[:, t0:t0 + cnt, :],
                                  wt.ap()[:, t0:t0 + cnt, :])

            nsteps = max(PAIR_STEPS)
            beta = [None, None, None]       # per-pair PSUM [W, 2*BC] f32
            csb = [None, None, None]        # per-pair SBUF c [W, 2*BC] bf16
            em_blk = [None, None, None]
            junc_row = [0]
            junc_sbs = []

            def em_slice(pi, i):
                j = i % TBLK
                if j == 0:
                    eblk = sb.tile([W, TBLK, 2 * BC], bf16,
                                   tag=f"em{pi}", bufs=3)
                    em_blk[pi] = eblk
                    i1 = min(i + TBLK, PAIR_STEPS[pi])
                    nc.sync.dma_start(eblk[:, :i1 - i, :],
                                      ems[pi].ap()[:, i:i1, :])
                return em_blk[pi][:, j, :]

            def junction_sum(c_half):
                # J = 1^T c  -> PSUM [1, BC] -> SBUF row, DMA'd out at end
                j_ps = ps.tile([1, BC], f32, tag="junc", bufs=2)
                nc.tensor.matmul(j_ps, ones_sb, c_half, start=True, stop=True)
                j_sb = sb.tile([1, BC], f32, tag=f"jsb{junc_row[0]}", bufs=1)
                nc.vector.tensor_copy(j_sb, j_ps)
                junc_sbs.append((junc_row[0], j_sb))
                junc_row[0] += 1

            for i in range(nsteps):
                for pi, (sa, sb_id) in enumerate(PAIRS):
                    if i >= PAIR_STEPS[pi]:
                        continue
                    lane = PAIR_LANE[pi]
                    em_i = em_slice(pi, i)
                    if i == 0:
                        c = em_i           # beta = 1 at chain start
                    else:
                        c = sb.tile([W, 2 * BC], bf16, tag=f"c{pi}", bufs=2)
                        if lane == "direct":
                            nc.vector.tensor_mul(c, beta[pi], em_i)
                        else:
                            bsb = sb.tile([W, 2 * BC], bf16, tag=f"bs{pi}",
                                          bufs=2)
                            nc.scalar.activation(bsb, beta[pi], Copy)
                            nc.vector.tensor_mul(c, bsb, em_i)
                    csb[pi] = c
                    if any(CHAIN_THI[s] - i > CHAIN_TLO[s]
                           for s in (sa, sb_id)):
                        bnew = ps.tile([W, 2 * BC], f32,
                                       tag=f"b{pi}", bufs=1)
                        beta[pi] = bnew
                    for h, s in enumerate((sa, sb_id)):
                        t = CHAIN_THI[s] - i
                        c_half = c[:, h * BC:(h + 1) * BC]
                        if t == CHAIN_TLO[s]:
                            junction_sum(c_half)     # last c of this chain
                        elif t == M_BOUND[s] and s < 6:
                            junction_sum(c_half)     # burn-in end J
                        if t > CHAIN_TLO[s]:
                            nc.tensor.matmul(
                                beta[pi][:, h * BC:(h + 1) * BC],
                                wt_sb[:, t - 1, :], c_half,
                                start=True, stop=True)

            assert junc_row[0] == 11, junc_row[0]
            for row, j_sb in junc_sbs:
                nc.sync.dma_start(juncs.ap()[row:row + 1, :], j_sb)
    nc.compile()
    return nc


def _host_prep(data, input_distros, dense_layer_weights):
    f64 = np.float64
    we = np.exp(dense_layer_weights.astype(f64))           # (255,W,W)
    rowsum = we.sum(axis=2)                                # (255,W)
    recip = 1.0 / rowsum
    d = input_distros.astype(f64)
    d = d - d.max(axis=1, keepdims=True)
    e = np.exp(d)
    Ll = e / e.sum(axis=1, keepdims=True)                  # (W,NB)
    bins = np.minimum(NB - 1, np.floor(
        data / np.float32(0.1)).astype(np.int32))          # (B,L)

    # column-0 f64 backward pass -> per-step rescale g_t, offset Cacc
    beta = np.ones(W, dtype=f64)
    Cacc = 0.0
    g = np.ones(L, dtype=f64)
    for t in range(L - 1, 0, -1):
        c = Ll[np.arange(W), bins[0, t]] * beta * recip[t - 1]
        tmp = we[t - 1].T @ c
        f = tmp.max()
        g[t] = 1.0 / f
        Cacc += np.log(f)
        beta = tmp * g[t]

    rsg = np.ones((W, L), dtype=f64)
    rsg[:, 1:] = recip.T * g[None, 1:]
    Lsc = (Ll[:, None, :] * rsg[:, :, None]).astype(np.float32)  # (W,L,NB)

    wt = np.ascontiguousarray(
        we.transpose(1, 0, 2)).astype(ml_dtypes.bfloat16)  # (W,255,W)
    ones_v = np.ones((W, 1), dtype=ml_dtypes.bfloat16)
    return wt, Lsc, bins, ones_v, Cacc


def _build_em_pair(Lsc, bins_core, pi):
    """em stream for pair pi: [W, steps, 2*BC] bf16.
    em[:, i, h*BC:(h+1)*BC] = Lsc[:, t, bins[:, t]] for chain h at step i."""
    steps = PAIR_STEPS[pi]
    out = np.empty((W, steps, 2 * BC), dtype=ml_dtypes.bfloat16)
    for h, s in enumerate(PAIRS[pi]):
        ts = CHAIN_THI[s] - np.arange(steps)                # (steps,)
        # gather: for each step i: Lsc[:, ts[i], bins_core[:, ts[i]]]
        bsel = bins_core[:, ts]                             # (BC, steps)
        blk = Lsc[:, ts[:, None], bsel.T]                   # (W, steps, BC)
        out[:, :, h * BC:(h + 1) * BC] = blk.astype(ml_dtypes.bfloat16)
    return out


def kernel(data, input_distros, dense_layer_weights):
    global LAST_LNY, LAST_RESULTS, _CACHED
    wt, Lsc, bins, ones_v, Cacc = _host_prep(
        np.asarray(data), np.asarray(input_distros),
        np.asarray(dense_layer_weights))

    if _CACHED is None:
        _CACHED = _build_nc()
    nc = _CACHED

    in_maps = []
    for cid in range(NCORES):
        bins_core = bins[cid * BC:(cid + 1) * BC, :]        # (BC, L)
        m = {"wt": wt, "ones": ones_v}
        for pi in range(3):
            m[f"em{pi}"] = _build_em_pair(Lsc, bins_core, pi)
        in_maps.append(m)

    res = run_bass_kernel_spmd(
        nc, in_maps, core_ids=list(range(NCORES)),
        trace=bool(int(os.environ.get("KERNEL_TRACE", "0"))))
    LAST_RESULTS = res

    # rows: emitted in program order per junction_sum() calls:
    #  pair0 chains (1,2) then pair1 (3,4) then pair2 (5,6), interleaved by
    #  step; mapping below reconstructs which row is which junction sum.
    rows = {}
    ri = 0
    for i in range(max(PAIR_STEPS)):
        for pi, (sa, sb_id) in enumerate(PAIRS):
            if i >= PAIR_STEPS[pi]:
                continue
            for s in (sa, sb_id):
                t = CHAIN_THI[s] - i
                if t == CHAIN_TLO[s]:
                    rows[("lo", s)] = ri; ri += 1
                elif t == M_BOUND[s] and s < 6:
                    rows[("burn", s)] = ri; ri += 1
    assert ri == 11, ri

    lnY = np.zeros(B, dtype=np.float64)
    for cid in range(NCORES):
        jr = LAST_RESULTS.results[cid]["juncs"].astype(np.float64)
        acc = np.log(jr[rows[("lo", 1)]])                   # colsum at t=0
        for s in range(1, 6):
            acc += np.log(jr[rows[("lo", s + 1)]])          # J_upper
            acc -= np.log(jr[rows[("burn", s)]])            # J_burn-in
        lnY[cid * BC:(cid + 1) * BC] = acc + Cacc
    LAST_LNY = lnY
    y = np.exp(lnY).astype(np.float32).reshape(B, 1)
    return y
